# revision 15
# baseline (speedup 1.0000x reference)
"""Trainium2 Bass kernel for nn_EncoderStack (B=4, T=2048, D=512, H=8, dk=dv=64, FF=2048).

Sharding: 8 cores = 4 batches x 2 head-groups. Core c handles batch b=c//2 and
the 4 heads hg*4..hg*4+3 where hg=c%2. The reference's raw reshape
(B,H,T,dv)->(B,T,H*dv) makes output rows i = h*256 + g depend only on head h
(row g of reshape(att[b,h], (256, 512)) with columns (r=t%8, j)). So the whole
post-attention chain (Wo + FFN) is per-(b,h) row-parallel: zero collectives.

Per-core pipeline (all on-chip, scores never touch HBM):
  QKV (f32r matmuls)  ->  per-head flash-ish attention:
    S = qT.T @ kT (f32r, scale folded into Wq on host), rowmax (DVE, negated),
    P = exp(S - max) (ACT, fp16 out, accum Z), P^T via DMA xbar transpose,
    att = PT.T @ v16 (fp16), att/Z, PE-transpose -> attT,
    A^T assembly (strided copies), Wo/FFN in fp16 -> yT out.
fp32r gives ~13-14 mantissa bits at bf16 speed; scores need ~0.01 abs accuracy
(softmax logits have std ~194 and near-argmax behaviour) which f32r meets.
"""
import os
import sys

sys.path.insert(0, "/opt/trn_rl_repo")

import numpy as np
import ml_dtypes

import concourse.bass as bass
import concourse.mybir as mybir
import concourse.tile as tile
from concourse import bass_utils
from concourse.masks import make_identity

# problem dims (hardcoded per contract)
B, T, D = 4, 2048, 512
H, DK, DV = 8, 64, 64
FF = 2048
NCORES = 8
NHL = 4           # heads per core
P_ = 128
NT = T // P_      # 16 q/s tiles
KT = D // P_      # 4 contraction tiles over D
FKT = FF // P_    # 16 contraction tiles over FF
G = T // 8        # 256 rows per head after the raw reshape

f32 = mybir.dt.float32
f32r = mybir.dt.float32r
f16 = mybir.dt.float16


def build_nc():
    nc = bass.Bass("TRN2", target_bir_lowering=False, debug=False,
                   detect_race_conditions=False)

    xT = nc.dram_tensor("xT", [D, T], f32, kind="ExternalInput")
    xTr = nc.dram_tensor("xTr", [D, T], f32r, kind="ExternalInput")
    wq = nc.dram_tensor("wq", [2, D, P_], f32, kind="ExternalInput")   # pairs, pre-scaled 1/8
    wk = nc.dram_tensor("wk", [2, D, P_], f32, kind="ExternalInput")
    wv = nc.dram_tensor("wv", [D, NHL * DV], f32r, kind="ExternalInput")
    wo = nc.dram_tensor("wo", [D, D], f16, kind="ExternalInput")
    w1 = nc.dram_tensor("w1", [D, FF], f32r, kind="ExternalInput")
    w2 = nc.dram_tensor("w2", [FF, D], f16, kind="ExternalInput")
    b1 = nc.dram_tensor("b1", [P_, FKT], f32, kind="ExternalInput")     # b1[mt*128+p] at [p, mt]
    b2 = nc.dram_tensor("b2", [P_, KT], f32, kind="ExternalInput")
    yT = nc.dram_tensor("yT", [NHL, D, G], f32, kind="ExternalOutput")
    kdbg = bool(os.environ.get("KDBG"))
    if kdbg:
        dq = nc.dram_tensor("dq", [P_, 2, T], f16, kind="ExternalOutput")
        dk_ = nc.dram_tensor("dk", [P_, 2, T], f16, kind="ExternalOutput")
        datt = nc.dram_tensor("datt", [DK, T], f16, kind="ExternalOutput")
        dout = nc.dram_tensor("dout", [P_, KT, G], f32r, kind="ExternalOutput")
        dp = nc.dram_tensor("dp", [P_, T], f16, kind="ExternalOutput")

    with tile.TileContext(nc) as tc:
        with tc.tile_pool(name="wp", bufs=1) as wp, \
             tc.tile_pool(name="qk", bufs=1) as qkp, \
             tc.tile_pool(name="hp", bufs=1) as hp, \
             tc.tile_pool(name="pp", bufs=2) as ppool, \
             tc.tile_pool(name="ptp", bufs=2) as ptp, \
             tc.tile_pool(name="sm", bufs=4) as smp, \
             tc.tile_pool(name="ysp", bufs=2) as ysp:

            # ---- persistent loads ----
            xsb = wp.tile([P_, KT, T], f32)
            nc.sync.dma_start(xsb[:], xT.ap().rearrange("(kt p) t -> p kt t", p=P_))
            xsbr = wp.tile([P_, KT, T], f32r)
            nc.sync.dma_start(xsbr[:], xTr.ap().rearrange("(kt p) t -> p kt t", p=P_))
            wqsb = wp.tile([P_, 2, KT, P_], f32)
            nc.sync.dma_start(wqsb[:], wq.ap().rearrange("pr (kt p) m -> p pr kt m", p=P_))
            wksb = wp.tile([P_, 2, KT, P_], f32)
            nc.sync.dma_start(wksb[:], wk.ap().rearrange("pr (kt p) m -> p pr kt m", p=P_))
            wvsb = wp.tile([P_, KT, NHL * DV], f32r)
            nc.sync.dma_start(wvsb[:], wv.ap().rearrange("(kt p) m -> p kt m", p=P_))
            wosb = wp.tile([P_, KT, D], f16)
            nc.sync.dma_start(wosb[:], wo.ap().rearrange("(kt p) m -> p kt m", p=P_))
            w1sb = wp.tile([P_, KT, FF], f32r)
            nc.sync.dma_start(w1sb[:], w1.ap().rearrange("(kt p) m -> p kt m", p=P_))
            w2sb = wp.tile([P_, FKT, D], f16)
            nc.sync.dma_start(w2sb[:], w2.ap().rearrange("(kt p) m -> p kt m", p=P_))
            b1sb = wp.tile([P_, FKT], f32)
            nc.sync.dma_start(b1sb[:], b1.ap())
            b2sb = wp.tile([P_, KT], f32)
            nc.sync.dma_start(b2sb[:], b2.ap())
            ident = wp.tile([P_, P_], f16)
            make_identity(nc, ident[:])

            qh_t = qkp.tile([P_, 2, T], f16)
            ql_t = qkp.tile([P_, 2, T], f16)
            kh_t = qkp.tile([P_, 2, T], f16)
            kl_t = qkp.tile([P_, 2, T], f16)
            v16 = qkp.tile([P_, NT, NHL * DV], f16)

            # ---- QKV projections ----
            with tc.tile_pool(name="qps", bufs=2, space="PSUM") as qps, \
                 tc.tile_pool(name="vps", bufs=2, space="PSUM") as vps:
                for wsb, hi_t, lo_t in ((wqsb, qh_t, ql_t), (wksb, kh_t, kl_t)):
                    for pr in range(2):
                        for half in range(2):
                            qp = qps.tile([P_, T // 2], f32, name="qp", tag="qkpsum")
                            for c in range(2):
                                off = half * (T // 2) + c * 512
                                for kt in range(KT):
                                    nc.tensor.matmul(
                                        qp[:, c * 512:(c + 1) * 512],
                                        wsb[:, pr, kt, :],
                                        xsb[:, kt, off:off + 512],
                                        start=(kt == 0), stop=(kt == KT - 1))
                            hsl = hi_t[:, pr, half * (T // 2):(half + 1) * (T // 2)]
                            nc.vector.tensor_copy(hsl, qp[:])
                            # lo = psum - hi (captures next 11 mantissa bits)
                            nc.vector.scalar_tensor_tensor(
                                lo_t[:, pr, half * (T // 2):(half + 1) * (T // 2)],
                                qp[:], 1.0, hsl,
                                op0=mybir.AluOpType.mult,
                                op1=mybir.AluOpType.subtract)
                for tt in range(NT):
                    vp = vps.tile([P_, NHL * DV], f32, name=f"vp{tt}", tag="vpsum")
                    for kt in range(KT):
                        nc.tensor.matmul(vp[:], xsbr[:, kt, tt * P_:(tt + 1) * P_],
                                         wvsb[:, kt, :],
                                         start=(kt == 0), stop=(kt == KT - 1))
                    nc.scalar.copy(v16[:, tt, :], vp[:])

            if kdbg:
                nc.sync.dma_start(dq.ap(), qh_t[:])
                nc.sync.dma_start(dk_.ap(), kh_t[:])
            # ---- per-head attention + Wo + FFN ----
            with tc.tile_pool(name="sps", bufs=2, space="PSUM") as sps, \
                 tc.tile_pool(name="avps", bufs=1, space="PSUM") as avps, \
                 tc.tile_pool(name="atps", bufs=1, space="PSUM") as atps, \
                 tc.tile_pool(name="ffps", bufs=2, space="PSUM") as ffps:
                for h in range(NHL):
                    pr, sl = h // 2, h % 2
                    p0, p1 = sl * DK, (sl + 1) * DK
                    attT = hp.tile([DK, T], f16, name=f"attT{h}", tag="attT")
                    for qt in range(NT):
                        qsl = (slice(p0, p1), pr, slice(qt * P_, (qt + 1) * P_))
                        qh_l, ql_l = qh_t[qsl], ql_t[qsl]
                        sh = []
                        for half in range(2):
                            st_ = sps.tile([P_, T // 2], f32, name=f"s{h}_{qt}_{half}", tag="spsum")
                            for c in range(2):
                                off = half * (T // 2) + c * 512
                                ks = (slice(p0, p1), pr, slice(off, off + 512))
                                dst = st_[:, c * 512:(c + 1) * 512]
                                nc.tensor.matmul(dst, qh_l, kh_t[ks], start=True, stop=False)
                                nc.tensor.matmul(dst, qh_l, kl_t[ks], start=False, stop=False)
                                nc.tensor.matmul(dst, ql_l, kh_t[ks], start=False, stop=True)
                            sh.append(st_)
                        nm0 = smp.tile([P_, 1], f32, name="nm0", tag="nm0")
                        nm1 = smp.tile([P_, 1], f32, name="nm1", tag="nm1")
                        nc.vector.tensor_reduce(nm0[:], sh[0][:], mybir.AxisListType.X,
                                                mybir.AluOpType.max, negate=True)
                        nc.vector.tensor_reduce(nm1[:], sh[1][:], mybir.AxisListType.X,
                                                mybir.AluOpType.max, negate=True)
                        nm = smp.tile([P_, 1], f32, name="nm", tag="nm")
                        nc.vector.tensor_tensor(nm[:], nm0[:], nm1[:], mybir.AluOpType.min)
                        ptile = ppool.tile([P_, T], f16, name="ptile", tag="ptile")
                        z0 = smp.tile([P_, 1], f32, name="z0", tag="z0")
                        z1 = smp.tile([P_, 1], f32, name="z1", tag="z1")
                        for half, zz in ((0, z0), (1, z1)):
                            nc.scalar.activation(
                                ptile[:, half * (T // 2):(half + 1) * (T // 2)],
                                sh[half][:], mybir.ActivationFunctionType.Exp,
                                bias=nm[:], scale=1.0, accum_out=zz[:])
                        z = smp.tile([P_, 1], f32, name="z", tag="z")
                        nc.vector.tensor_tensor(z[:], z0[:], z1[:], mybir.AluOpType.add)
                        rz = smp.tile([P_, 1], f32, name="rz", tag="rz")
                        nc.vector.reciprocal(rz[:], z[:])
                        if kdbg and h == 0 and qt == int(os.environ.get("KDBG_QT", "0")):
                            nc.sync.dma_start(dp.ap(), ptile[:])
                        pt = ptp.tile([P_, NT, P_], f16, name="pt", tag="pt")
                        nc.sync.dma_start_transpose(pt[:, 0:NT // 2, :], ptile[:, 0:T // 2])
                        nc.sync.dma_start_transpose(pt[:, NT // 2:NT, :], ptile[:, T // 2:T])
                        att = avps.tile([P_, DK], f32, name="att", tag="att")
                        for st in range(NT):
                            nc.tensor.matmul(att[:], pt[:, st, :],
                                             v16[:, st, h * DV:(h + 1) * DV],
                                             start=(st == 0), stop=(st == NT - 1))
                        att16 = smp.tile([P_, DK], f16, name="att16", tag="att16")
                        nc.vector.tensor_scalar_mul(att16[:], att[:], rz[:])
                        atp = atps.tile([DK, P_], f16, name="atp", tag="atp")
                        nc.tensor.transpose(atp[:], att16[:], ident[:])
                        if qt % 2 == 0:
                            nc.vector.tensor_copy(attT[:, qt * P_:(qt + 1) * P_], atp[:])
                        else:
                            nc.scalar.copy(attT[:, qt * P_:(qt + 1) * P_], atp[:])
                    if kdbg and h == 0:
                        nc.sync.dma_start(datt.ap(), attT[:])
                    # A^T assembly: aT[(r%2)*64+j, r//2, g] = attT[j, g*8+r]
                    aT = hp.tile([P_, KT, G], f16, name=f"aT{h}", tag="aT")
                    attT_r = attT.rearrange("j (g r) -> j r g", r=8)
                    for r in range(8):
                        nc.gpsimd.tensor_copy(
                            aT[(r % 2) * DK:(r % 2 + 1) * DK, r // 2, :],
                            attT_r[:, r, :])
                    # Wo: outT[mt] = (A @ Wo)^T chunk
                    outT = hp.tile([P_, KT, G], f32r, name=f"outT{h}", tag="outT")
                    for mt in range(KT):
                        ops = ffps.tile([P_, G], f32, name="ops", tag="ff")
                        for kt in range(KT):
                            nc.tensor.matmul(ops[:], wosb[:, kt, mt * P_:(mt + 1) * P_],
                                             aT[:, kt, :],
                                             start=(kt == 0), stop=(kt == KT - 1))
                        nc.scalar.copy(outT[:, mt, :], ops[:])
                    if kdbg and h == 0:
                        nc.sync.dma_start(dout.ap(), outT[:])
                    # FFN: hT = relu(W1^T @ outT + b1), yT = W2^T @ hT + b2
                    hT = hp.tile([P_, FKT, G], f16, name=f"hT{h}", tag="hT")
                    for mt in range(FKT):
                        hps = ffps.tile([P_, G], f32, name="hps", tag="ff")
                        for kt in range(KT):
                            nc.tensor.matmul(hps[:], w1sb[:, kt, mt * P_:(mt + 1) * P_],
                                             outT[:, kt, :],
                                             start=(kt == 0), stop=(kt == KT - 1))
                        nc.scalar.activation(hT[:, mt, :], hps[:],
                                             mybir.ActivationFunctionType.Relu,
                                             bias=b1sb[:, mt:mt + 1], scale=1.0)
                    for mt in range(KT):
                        yps = ffps.tile([P_, G], f32, name="yps", tag="ff")
                        for kt in range(FKT):
                            nc.tensor.matmul(yps[:], w2sb[:, kt, mt * P_:(mt + 1) * P_],
                                             hT[:, kt, :],
                                             start=(kt == 0), stop=(kt == FKT - 1))
                        ysb = ysp.tile([P_, G], f32, name="ysb", tag="ysb")
                        nc.vector.tensor_scalar_add(ysb[:], yps[:], b2sb[:, mt:mt + 1])
                        nc.sync.dma_start(yT.ap()[h, mt * P_:(mt + 1) * P_, :], ysb[:])
    return nc


def split_sync_waits(nc, maxw: int = 1):
    """The installed walrus rejects >1 sem wait per instruction ("Too many sync
    wait commands"). Move overflow waits onto preceding same-engine NoOps —
    engine program order is serial, so semantics are identical."""
    import copy as _copy
    fn = nc.m.functions[0]
    ctr = 0
    new_blocks = []
    changed = False
    for blk in fn.blocks:
        new_insts = []
        blk_changed = False
        for inst in blk.instructions:
            si = inst.sync_info
            if si is not None and si.on_wait is not None and len(si.on_wait) > maxw:
                waits = list(si.on_wait)
                while len(waits) > maxw:
                    chunk, waits = waits[:maxw], waits[maxw:]
                    ctr += 1
                    nop = mybir.InstNoOp(
                        name=f"I-waitsplit-{ctr}", engine=inst.engine, ins=[], outs=[])
                    nop.sync_info = mybir.SyncInfo(on_wait=chunk, on_update=[])
                    new_insts.append(nop)
                si.on_wait = waits
                inst.sync_info = si
                blk_changed = True
            new_insts.append(inst)
        if blk_changed:
            changed = True
            new_blocks.append(_copy.replace(blk, instructions=new_insts))
        else:
            new_blocks.append(blk)
    if changed:
        new_fn = _copy.replace(fn, blocks=[])
        new_fn.set_allocations_from_list(fn.allocations)
        for nb in new_blocks:
            new_fn.blocks.append(nb)
        new_m = _copy.replace(nc.m, functions=[])
        new_m.functions.append(new_fn)
        for f in nc.m.functions[1:]:
            new_m.functions.append(f)
        nc.m = new_m
    return ctr


_NC_CACHE = None


def _get_nc():
    global _NC_CACHE
    if _NC_CACHE is None:
        nc = build_nc()
        split_sync_waits(nc, maxw=1)
        _NC_CACHE = nc
    return _NC_CACHE


def _prep_core_inputs(x, Wq, Wk, Wv, Wo, W1, b1, W2, b2, core):
    b, hg = core // 2, core % 2
    heads = range(hg * NHL, (hg + 1) * NHL)
    h0 = hg * NHL
    xT = np.ascontiguousarray(x[b].T).astype(np.float32)
    wq_pairs = np.stack([
        np.concatenate([Wq[h0 + 2 * pr] * 0.125, Wq[h0 + 2 * pr + 1] * 0.125], axis=1)
        for pr in range(2)])
    wk_pairs = np.stack([
        np.concatenate([Wk[h0 + 2 * pr], Wk[h0 + 2 * pr + 1]], axis=1)
        for pr in range(2)])
    wv_c = np.concatenate([Wv[h] for h in heads], axis=1)
    return {
        "xT": xT,
        "xTr": xT,
        "wq": wq_pairs.astype(np.float32),
        "wk": wk_pairs.astype(np.float32),
        "wv": wv_c.astype(np.float32),
        "wo": Wo.astype(np.float16),
        "w1": W1.astype(np.float32),
        "w2": W2.astype(np.float16),
        "b1": np.ascontiguousarray(b1.reshape(FKT, P_).T).astype(np.float32),
        "b2": np.ascontiguousarray(b2.reshape(KT, P_).T).astype(np.float32),
    }


def kernel(x, Wq, Wk, Wv, Wo, W1, b1, W2, b2, _trace=False):
    x = np.asarray(x, dtype=np.float32)
    Wq = np.asarray(Wq, dtype=np.float32)
    Wk = np.asarray(Wk, dtype=np.float32)
    Wv = np.asarray(Wv, dtype=np.float32)
    Wo = np.asarray(Wo, dtype=np.float32)
    W1 = np.asarray(W1, dtype=np.float32)
    b1 = np.asarray(b1, dtype=np.float32)
    W2 = np.asarray(W2, dtype=np.float32)
    b2 = np.asarray(b2, dtype=np.float32)

    nc = _get_nc()
    in_maps = [_prep_core_inputs(x, Wq, Wk, Wv, Wo, W1, b1, W2, b2, c)
               for c in range(NCORES)]
    res = bass_utils.run_bass_kernel_spmd(
        nc, in_maps, core_ids=list(range(NCORES)), trace=_trace)

    y = np.empty((B, T, D), dtype=np.float32)
    for c in range(NCORES):
        b, hg = c // 2, c % 2
        yt = res.results[c]["yT"]          # [NHL, D, G]
        for l in range(NHL):
            h = hg * NHL + l
            y[b, h * G:(h + 1) * G, :] = yt[l].T
    if _trace:
        return y, res
    return y
